# revision 1
# baseline (speedup 1.0000x reference)
"""GCN 2-layer (PyG GCNConv x2 + ReLU) Bass kernel for Trainium2, 8-core SPMD.

Strategy:
  - Host: add self-loops, compute symmetric normalization dinv = deg^-1/2,
    fold dinv[src] into a prescaled gather table (x * dinv), shard dst nodes
    contiguously across 8 cores, sort each core's edges by dst into 128-node
    "windows", pack edges into 128-edge "chunks" (one matmul each).
    dma_gather uses int16 indices, so the node table is addressed via two
    32768-row views (LOW/HIGH); each window's edges are split into LOW chunks
    and HIGH chunks, and the kernel runs all LOW chunks (accumulating per
    window in PSUM, evicting to SBUF), then all HIGH chunks (added on top).
  - Device per core:
      Phase A (layer 1): dma_gather source rows of the prescaled x-table ->
        G [128e, d_in]; build one-hot S [128e, 128dst] on DVE (iota ==
        dst_rel); PE matmul accumulates G.T @ S into PSUM [d_in, 128dst]
        per window (aggregated x per dst, transposed).  Per window: x W1
        (PE), scale by dinv[dst], +b1, ReLU; transpose (PE); x W2; scale by
        dinv[dst]; replicate 32x -> 256B rows of the h2 table, DMA out.
      AllGather h2 shards -> full [N, 64] table.
      Phase B (layer 2): same chunk structure; gather h2 rows, matmul
        S.T @ G2[:, :2] accumulated per window; scale by dinv[dst], +b2.
"""

import numpy as np

import concourse.bass as bass
import concourse.mybir as mybir
import concourse.tile as tile
from concourse import bacc
from concourse.bass_utils import run_bass_kernel_spmd

F32 = mybir.dt.float32
BF16 = mybir.dt.bfloat16
I16 = mybir.dt.int16

N_CORES = 8
WINDOW = 128  # dst nodes per PSUM accumulation window
CHUNK = 128  # edges per matmul chunk
GSZ = 8  # max chunks per dma_gather instruction (1024 idxs, single-packet)
SBATCH = 8  # chunks per S-build DVE op
HALF = 32768  # int16 index range
REP = 64  # h2 replication (64x2 bf16 cols -> 256B rows)
GATHER_BF16 = True  # layer-1 gather table + chunk matmuls in bf16


# --------------------------------------------------------------------------
# Host preprocessing
# --------------------------------------------------------------------------
def _preprocess(x, edge_index, n_cores):
    N = x.shape[0]
    src = np.concatenate(
        [np.asarray(edge_index[0], dtype=np.int64), np.arange(N, dtype=np.int64)]
    )
    dst = np.concatenate(
        [np.asarray(edge_index[1], dtype=np.int64), np.arange(N, dtype=np.int64)]
    )
    deg = np.bincount(dst, minlength=N).astype(np.float64)
    dinv = np.where(deg > 0, 1.0 / np.sqrt(deg), 0.0).astype(np.float32)

    n_local = (N + n_cores - 1) // n_cores
    w_cnt = (n_local + WINDOW - 1) // WINDOW

    order = np.argsort(dst, kind="stable")
    s_src = src[order]
    s_dst = dst[order]

    # table rows: 0 = zero, 1..N = nodes, N+1 = zero.  row(n) = n+1
    # LOW view = rows [0, min(HALF, N+2));  HIGH view = rows [HB, HB+HALF)
    HB = max(0, N + 2 - HALF)
    lowmax_row = min(HALF, N + 2)  # rows < this go to LOW chunks
    pad_low = 0  # zero row 0
    pad_high = N + 1 - HB  # zero row N+1 relative to HB

    # per (core, window): split edges into LOW (row < lowmax) and HIGH
    parts = {}  # (c, w, hi) -> (rows_arr, dstrel_arr)
    counts = np.zeros((2, n_cores, w_cnt), dtype=np.int64)
    for c in range(n_cores):
        base = c * n_local
        for w in range(w_cnt):
            wlo = base + w * WINDOW
            whi = min(base + (w + 1) * WINDOW, base + n_local, N)
            lo_i = np.searchsorted(s_dst, wlo, side="left")
            hi_i = np.searchsorted(s_dst, whi, side="left")
            rows = (s_src[lo_i:hi_i] + 1).astype(np.int64)
            rel = (s_dst[lo_i:hi_i] - wlo).astype(np.float32)
            is_lo = rows < lowmax_row
            parts[(c, w, 0)] = (rows[is_lo], rel[is_lo])
            parts[(c, w, 1)] = (rows[~is_lo] - HB, rel[~is_lo])
            counts[0, c, w] = is_lo.sum()
            counts[1, c, w] = (~is_lo).sum()

    # uniform per-window chunk counts across cores, per section
    kw_lo = np.maximum(1, np.ceil(counts[0] / CHUNK).astype(np.int64).max(axis=0))
    kw_hi = np.maximum(1, np.ceil(counts[1] / CHUNK).astype(np.int64).max(axis=0))
    T_lo, T_hi = int(kw_lo.sum()), int(kw_hi.sum())
    T = T_lo + T_hi

    # chunk order: LOW section (windows in order), then HIGH section
    chunk_win = []  # (window, first_in_sec, last_in_sec, section)
    for sec, kws in ((0, kw_lo), (1, kw_hi)):
        for w in range(w_cnt):
            for k in range(kws[w]):
                chunk_win.append((w, k == 0, k == kws[w] - 1, sec))

    per_core = []
    for c in range(n_cores):
        idx_lin = np.zeros(T * CHUNK, dtype=np.int32)
        dstrel = np.zeros((CHUNK, T), dtype=np.float32)
        t = 0
        for sec, kws, padrow in ((0, kw_lo, pad_low), (1, kw_hi, pad_high)):
            for w in range(w_cnt):
                rows, rel = parts[(c, w, sec)]
                n_e = len(rows)
                n_slots = int(kws[w]) * CHUNK
                buf = np.full(n_slots, padrow, dtype=np.int32)
                buf[:n_e] = rows
                idx_lin[t * CHUNK : t * CHUNK + n_slots] = buf
                rbuf = np.zeros(n_slots, dtype=np.float32)
                rbuf[:n_e] = rel
                dstrel[:, t : t + int(kws[w])] = rbuf.reshape(int(kws[w]), CHUNK).T
                t += int(kws[w])
        assert t == T
        # dma_gather idx layout: [128, T*8] int16; linear i = s*16 + r
        # (rows 0..15, replicated to all 128 partitions)
        idx16 = idx_lin.astype(np.int16).reshape(T * CHUNK // 16, 16).T  # [16, S]
        idx16 = np.tile(idx16, (8, 1))  # [128, S]

        dinvw = np.zeros((WINDOW, w_cnt), dtype=np.float32)
        base = c * n_local
        for w in range(w_cnt):
            wlo = base + w * WINDOW
            whi = min(wlo + WINDOW, base + n_local, N)
            if whi > wlo:
                dinvw[: whi - wlo, w] = dinv[wlo:whi]
        per_core.append({"idx16": idx16, "dstrel": dstrel, "dinvw": dinvw})

    return {
        "n_local": n_local,
        "w_cnt": w_cnt,
        "kw_lo": kw_lo,
        "kw_hi": kw_hi,
        "T_lo": T_lo,
        "T_hi": T_hi,
        "T": T,
        "HB": HB,
        "chunk_win": chunk_win,
        "dinv": dinv,
        "per_core": per_core,
    }


# --------------------------------------------------------------------------
# Device kernel builder (one program, SPMD across cores)
# --------------------------------------------------------------------------
def _build(nc, *, N, n_local, d_in, d_hid, n_cls, pp, n_cores, dt_gat):
    Relu = mybir.ActivationFunctionType.Relu
    Copy = mybir.ActivationFunctionType.Copy
    T, T_lo = pp["T"], pp["T_lo"]
    w_cnt, HB = pp["w_cnt"], pp["HB"]
    chunk_win = pp["chunk_win"]
    d_rep = REP * n_cls  # 64 cols of f32 -> 256B rows

    xtab = nc.dram_tensor("xtab", [N + 2, d_in], dt_gat, kind="ExternalInput")
    w1 = nc.dram_tensor("w1", [d_in, d_hid], F32, kind="ExternalInput")
    w2 = nc.dram_tensor("w2", [d_hid, n_cls], F32, kind="ExternalInput")
    b1bc = nc.dram_tensor("b1bc", [WINDOW, d_hid], F32, kind="ExternalInput")
    b2bc = nc.dram_tensor("b2bc", [WINDOW, n_cls], F32, kind="ExternalInput")
    iota = nc.dram_tensor("iota", [CHUNK, SBATCH * WINDOW], F32, kind="ExternalInput")
    ident = nc.dram_tensor("ident", [WINDOW, WINDOW], F32, kind="ExternalInput")
    idx_t = nc.dram_tensor("idx16", [CHUNK, T * 8], I16, kind="ExternalInput")
    dstrel_t = nc.dram_tensor("dstrel", [CHUNK, T], F32, kind="ExternalInput")
    dinvw_t = nc.dram_tensor("dinvw", [WINDOW, w_cnt], F32, kind="ExternalInput")
    out_t = nc.dram_tensor("out", [n_local, n_cls], F32, kind="ExternalOutput")

    h2loc = nc.dram_tensor("h2loc", [n_local, d_rep], BF16)
    h2tab = nc.dram_tensor("h2tab", [N + 2, d_rep], BF16, addr_space="Shared")

    # per-section gather groups: (sec, t0, n)
    groups = []
    for sec, tlo, thi in ((0, 0, T_lo), (1, T_lo, T)):
        t0 = tlo
        while t0 < thi:
            n = min(GSZ, thi - t0)
            groups.append((sec, t0, n))
            t0 += n

    def tab_view(tab):
        return [
            tab[0 : min(HALF, N + 2), :],
            tab[HB : min(HB + HALF, N + 2), :],
        ]

    with tile.TileContext(nc) as tc:
        with (
            tc.tile_pool(name="const", bufs=1) as cpool,
            tc.tile_pool(name="gbuf", bufs=3) as gpool,
            tc.tile_pool(name="g2buf", bufs=3) as g2pool,
            tc.tile_pool(name="sbat", bufs=3) as spool,
            tc.tile_pool(name="sbat2", bufs=3) as s2pool,
            tc.tile_pool(name="wtmp", bufs=3) as wpool,
            tc.tile_pool(name="aggs", bufs=1) as apool,
            tc.tile_pool(name="psA", bufs=3, space="PSUM") as psA,
            tc.tile_pool(name="psW", bufs=3, space="PSUM") as psW,
        ):
            # ---- constants into SBUF ----
            w1_sb = cpool.tile([d_in, d_hid], F32, tag="w1")
            nc.sync.dma_start(out=w1_sb[:], in_=w1[:])
            w2_sb = cpool.tile([d_hid, n_cls], F32, tag="w2")
            nc.sync.dma_start(out=w2_sb[:], in_=w2[:])
            b1_sb = cpool.tile([WINDOW, d_hid], F32, tag="b1")
            nc.sync.dma_start(out=b1_sb[:], in_=b1bc[:])
            b2_sb = cpool.tile([WINDOW, n_cls], F32, tag="b2")
            nc.sync.dma_start(out=b2_sb[:], in_=b2bc[:])
            iota_sb = cpool.tile([CHUNK, SBATCH * WINDOW], F32, tag="iota")
            nc.sync.dma_start(out=iota_sb[:], in_=iota[:])
            id_sb = cpool.tile([WINDOW, WINDOW], F32, tag="ident")
            nc.sync.dma_start(out=id_sb[:], in_=ident[:])
            idx_sb = cpool.tile([CHUNK, T * 8], I16, tag="idx")
            nc.sync.dma_start(out=idx_sb[:], in_=idx_t[:])
            dstrel_sb = cpool.tile([CHUNK, T], F32, tag="dstrel")
            nc.sync.dma_start(out=dstrel_sb[:], in_=dstrel_t[:])
            dinvw_sb = cpool.tile([WINDOW, w_cnt], F32, tag="dinvw")
            nc.sync.dma_start(out=dinvw_sb[:], in_=dinvw_t[:])

            zrow = cpool.tile([1, d_rep], BF16, tag="zrow")
            nc.vector.memset(zrow[:], 0.0)
            nc.sync.dma_start(out=h2tab[0:1, :], in_=zrow[:1, :])
            nc.sync.dma_start(out=h2tab[N + 1 : N + 2, :], in_=zrow[:1, :])

            def build_s(pool, t0, n, nm):
                """one-hot S for chunks [t0, t0+n) in one DVE op."""
                s_tile = pool.tile([CHUNK, SBATCH * WINDOW], BF16, tag="s", name=nm)
                rel_b = (
                    dstrel_sb[:, t0 : t0 + n]
                    .rearrange("p (b one) -> p b one", one=1)
                    .to_broadcast([CHUNK, n, WINDOW])
                )
                io_v = iota_sb[:, : n * WINDOW].rearrange("p (b j) -> p b j", j=WINDOW)
                s_v = s_tile[:, : n * WINDOW].rearrange("p (b j) -> p b j", j=WINDOW)
                nc.vector.tensor_tensor(
                    out=s_v, in0=io_v, in1=rel_b, op=mybir.AluOpType.is_equal
                )
                return s_tile

            # per-window accumulators in SBUF (LOW evicts, HIGH adds on top)
            aggT_sb = apool.tile([d_in, w_cnt * WINDOW], F32, tag="aggT")
            out2_sb = apool.tile([WINDOW, w_cnt * n_cls], F32, tag="out2")

            # =========================== PHASE A ===========================
            psum_of_win = {}
            for sec, t0, n in groups:
                gb = gpool.tile([CHUNK, GSZ, d_in], dt_gat, tag="g", name="gb")
                nc.gpsimd.dma_gather(
                    gb[:, :n, :],
                    tab_view(xtab)[sec],
                    idx_sb[:, t0 * 8 : (t0 + n) * 8],
                    n * CHUNK,
                    n * CHUNK,
                    d_in,
                    single_packet=True,
                )
                for bt0 in range(t0, t0 + n, SBATCH):
                    bn = min(SBATCH, t0 + n - bt0)
                    s_tile = build_s(spool, bt0, bn, "sA")
                    for t in range(bt0, bt0 + bn):
                        j = t - bt0
                        w, first, last, _sec = chunk_win[t]
                        if first:
                            psum_of_win[w] = psA.tile(
                                [d_in, WINDOW], F32, tag="agg", name="aggps"
                            )
                        nc.tensor.matmul(
                            out=psum_of_win[w][:],
                            lhsT=gb[:, t - t0, :],
                            rhs=s_tile[:, j * WINDOW : (j + 1) * WINDOW],
                            start=first,
                            stop=last,
                        )
                        if not last:
                            continue
                        ps = psum_of_win.pop(w)
                        wsl = aggT_sb[:, w * WINDOW : (w + 1) * WINDOW]
                        if _sec == 0:
                            nc.scalar.activation(out=wsl, in_=ps[:], func=Copy)
                        else:
                            nc.vector.tensor_tensor(
                                out=wsl, in0=ps[:], in1=wsl, op=mybir.AluOpType.add
                            )
                            _window_epilogue_A(
                                nc, w, wsl, wpool, psW, w1_sb, w2_sb, b1_sb,
                                dinvw_sb, id_sb, h2loc, n_local, d_in, d_hid,
                                n_cls, d_rep,
                            )

            # ======================= h2 exchange ==========================
            if n_cores > 1:
                nc.gpsimd.collective_compute(
                    "AllGather",
                    mybir.AluOpType.bypass,
                    replica_groups=[list(range(n_cores))],
                    ins=[h2loc[:]],
                    outs=[h2tab[1 : 1 + n_cores * n_local, :]],
                )
            else:
                nc.sync.dma_start(out=h2tab[1 : 1 + n_local, :], in_=h2loc[:])

            # =========================== PHASE B ===========================
            psum_of_win = {}
            for sec, t0, n in groups:
                g2 = g2pool.tile([CHUNK, GSZ, d_rep], BF16, tag="g2", name="g2b")
                nc.gpsimd.dma_gather(
                    g2[:, :n, :],
                    tab_view(h2tab)[sec],
                    idx_sb[:, t0 * 8 : (t0 + n) * 8],
                    n * CHUNK,
                    n * CHUNK,
                    d_rep,
                    single_packet=True,
                )
                for bt0 in range(t0, t0 + n, SBATCH):
                    bn = min(SBATCH, t0 + n - bt0)
                    s_tile = build_s(s2pool, bt0, bn, "sB")
                    for t in range(bt0, bt0 + bn):
                        j = t - bt0
                        w, first, last, _sec = chunk_win[t]
                        if first:
                            psum_of_win[w] = psA.tile(
                                [WINDOW, n_cls], F32, tag="agg", name="agg2ps"
                            )
                        nc.tensor.matmul(
                            out=psum_of_win[w][:],
                            lhsT=s_tile[:, j * WINDOW : (j + 1) * WINDOW],
                            rhs=g2[:, t - t0, :n_cls],
                            start=first,
                            stop=last,
                        )
                        if not last:
                            continue
                        ps = psum_of_win.pop(w)
                        osl = out2_sb[:, w * n_cls : (w + 1) * n_cls]
                        if _sec == 0:
                            nc.scalar.activation(out=osl, in_=ps[:], func=Copy)
                        else:
                            ob = wpool.tile([WINDOW, n_cls], F32, tag="ob")
                            nc.vector.tensor_tensor(
                                out=ob[:], in0=ps[:], in1=osl, op=mybir.AluOpType.add
                            )
                            ob2 = wpool.tile([WINDOW, n_cls], F32, tag="ob2")
                            nc.vector.tensor_scalar(
                                out=ob2[:],
                                in0=ob[:],
                                scalar1=dinvw_sb[:, w : w + 1],
                                scalar2=None,
                                op0=mybir.AluOpType.mult,
                            )
                            ob3 = wpool.tile([WINDOW, n_cls], F32, tag="ob3")
                            nc.vector.tensor_tensor(
                                out=ob3[:], in0=ob2[:], in1=b2_sb[:],
                                op=mybir.AluOpType.add,
                            )
                            nrows = min(WINDOW, n_local - w * WINDOW)
                            nc.sync.dma_start(
                                out=out_t[w * WINDOW : w * WINDOW + nrows, :],
                                in_=ob3[:nrows, :],
                            )

    nc.compile()
    return nc


def _window_epilogue_A(
    nc, w, aggT, wpool, psW, w1_sb, w2_sb, b1_sb, dinvw_sb, id_sb,
    h2loc, n_local, d_in, d_hid, n_cls, d_rep,
):
    """aggT [d_in, WINDOW] in SBUF -> replicated h2 rows in DRAM."""
    Relu = mybir.ActivationFunctionType.Relu
    Copy = mybir.ActivationFunctionType.Copy

    # h1 [dst, hid] = aggT.T @ W1
    h1_ps = psW.tile([WINDOW, d_hid], F32, tag="wps", name="h1_ps")
    nc.tensor.matmul(out=h1_ps[:], lhsT=aggT, rhs=w1_sb[:], start=True, stop=True)
    # scale by dinv[dst] (per-partition), + b1, relu
    r_sb = wpool.tile([WINDOW, d_hid], F32, tag="r")
    nc.vector.tensor_scalar(
        out=r_sb[:],
        in0=h1_ps[:],
        scalar1=dinvw_sb[:, w : w + 1],
        scalar2=None,
        op0=mybir.AluOpType.mult,
    )
    r2_sb = wpool.tile([WINDOW, d_hid], F32, tag="r2")
    nc.vector.tensor_tensor(
        out=r2_sb[:], in0=r_sb[:], in1=b1_sb[:], op=mybir.AluOpType.add
    )
    r3_sb = wpool.tile([WINDOW, d_hid], F32, tag="r3")
    nc.scalar.activation(out=r3_sb[:], in_=r2_sb[:], func=Relu)
    # transpose -> [hid, dst]
    rT_ps = psW.tile([d_hid, WINDOW], F32, tag="wps", name="rT_ps")
    nc.tensor.transpose(out=rT_ps[:], in_=r3_sb[:], identity=id_sb[:])
    rT_sb = wpool.tile([d_hid, WINDOW], F32, tag="rTs")
    nc.scalar.activation(out=rT_sb[:], in_=rT_ps[:], func=Copy)
    # h2 [dst, n_cls] = rT.T @ W2; scale by dinv[dst]; replicate REP x
    h2_ps = psW.tile([WINDOW, n_cls], F32, tag="wps", name="h2_ps")
    nc.tensor.matmul(out=h2_ps[:], lhsT=rT_sb[:], rhs=w2_sb[:], start=True, stop=True)
    h2_sb = wpool.tile([WINDOW, d_rep], BF16, tag="h2s")
    nc.vector.tensor_scalar(
        out=h2_sb[:].rearrange("p (r c) -> p r c", c=n_cls),
        in0=h2_ps[:]
        .rearrange("p (one c) -> p one c", one=1)
        .to_broadcast([WINDOW, REP, n_cls]),
        scalar1=dinvw_sb[:, w : w + 1],
        scalar2=None,
        op0=mybir.AluOpType.mult,
    )
    nrows = min(WINDOW, n_local - w * WINDOW)
    nc.sync.dma_start(
        out=h2loc[w * WINDOW : w * WINDOW + nrows, :], in_=h2_sb[:nrows, :]
    )


# --------------------------------------------------------------------------
# Entry point
# --------------------------------------------------------------------------
def _make_inputs(x, W1, b1, W2, b2, pp, dt_np):
    N, d_in = x.shape
    W1 = np.asarray(W1, np.float32)
    b1 = np.asarray(b1, np.float32)
    W2 = np.asarray(W2, np.float32)
    b2 = np.asarray(b2, np.float32)
    d_hid = W1.shape[1]
    n_cls = W2.shape[1]
    xtab = np.concatenate(
        [
            np.zeros((1, d_in), np.float32),
            x * pp["dinv"][:, None],
            np.zeros((1, d_in), np.float32),
        ]
    ).astype(dt_np)
    iota_arr = np.broadcast_to(
        np.tile(np.arange(WINDOW, dtype=np.float32), SBATCH),
        (CHUNK, SBATCH * WINDOW),
    ).copy()
    shared = {
        "xtab": xtab,
        "w1": W1,
        "w2": W2,
        "b1bc": np.broadcast_to(b1, (WINDOW, d_hid)).astype(np.float32).copy(),
        "b2bc": np.broadcast_to(b2, (WINDOW, n_cls)).astype(np.float32).copy(),
        "iota": iota_arr,
        "ident": np.eye(WINDOW, dtype=np.float32),
    }
    in_maps = []
    for pc in pp["per_core"]:
        m = dict(shared)
        m["idx16"] = pc["idx16"]
        m["dstrel"] = pc["dstrel"]
        m["dinvw"] = pc["dinvw"]
        in_maps.append(m)
    return in_maps


def _run(x, edge_index, W1, b1, W2, b2, n_cores, trace=False):
    x = np.asarray(x, dtype=np.float32)
    N, d_in = x.shape
    d_hid = np.asarray(W1).shape[1]
    n_cls = np.asarray(W2).shape[1]
    assert d_in == 128 and d_hid == 128

    pp = _preprocess(x, edge_index, n_cores)
    dt_gat = BF16 if GATHER_BF16 else F32
    np_gat = np.dtype("bfloat16") if GATHER_BF16 else np.dtype("float32")

    nc = bacc.Bacc("TRN2", target_bir_lowering=False, debug=False)
    _build(
        nc,
        N=N,
        n_local=pp["n_local"],
        d_in=d_in,
        d_hid=d_hid,
        n_cls=n_cls,
        pp=pp,
        n_cores=n_cores,
        dt_gat=dt_gat,
    )

    import ml_dtypes  # noqa

    in_maps = _make_inputs(x, W1, b1, W2, b2, pp, np_gat)
    res = run_bass_kernel_spmd(nc, in_maps, list(range(n_cores)), trace=trace)
    outs = [res.results[c]["out"] for c in range(n_cores)]
    full = np.concatenate(outs, axis=0)[:N]
    return full.astype(np.float32), res


def kernel(x, edge_index, W1, b1, W2, b2):
    out, _ = _run(x, edge_index, W1, b1, W2, b2, N_CORES)
    return out



# revision 3
# speedup vs baseline: 1.2690x; 1.2690x over previous
"""GCN 2-layer (PyG GCNConv x2 + ReLU) Bass kernel for Trainium2, 8-core SPMD.

Strategy:
  - Host: add self-loops, compute symmetric normalization dinv = deg^-1/2,
    fold dinv[src] into a prescaled gather table (x * dinv), shard dst nodes
    contiguously across 8 cores, sort each core's edges by dst into 128-node
    "windows", pack edges into 128-edge "chunks" (one matmul each).
    dma_gather uses int16 indices, so the node table is addressed via two
    32768-row views (LOW/HIGH); each window's edges are split into LOW chunks
    and HIGH chunks, and the kernel runs all LOW chunks (accumulating per
    window in PSUM, evicting to SBUF), then all HIGH chunks (added on top).
  - Device per core:
      Phase A (layer 1): dma_gather source rows of the prescaled x-table ->
        G [128e, d_in]; build one-hot S [128e, 128dst] on DVE (iota ==
        dst_rel); PE matmul accumulates G.T @ S into PSUM [d_in, 128dst]
        per window (aggregated x per dst, transposed).  Per window: x W1
        (PE), scale by dinv[dst], +b1, ReLU; transpose (PE); x W2; scale by
        dinv[dst]; replicate 32x -> 256B rows of the h2 table, DMA out.
      AllGather h2 shards -> full [N, 64] table.
      Phase B (layer 2): same chunk structure; gather h2 rows, matmul
        S.T @ G2[:, :2] accumulated per window; scale by dinv[dst], +b2.
"""

import numpy as np

import concourse.bass as bass
import concourse.mybir as mybir
import concourse.tile as tile
from concourse import bacc
from concourse.bass_utils import run_bass_kernel_spmd

F32 = mybir.dt.float32
BF16 = mybir.dt.bfloat16
I16 = mybir.dt.int16

N_CORES = 8
WINDOW = 128  # dst nodes per PSUM accumulation window
CHUNK = 128  # edges per matmul chunk
GSZ = 8  # max chunks per dma_gather instruction (1024 idxs, single-packet)
SBATCH = 8  # chunks per S-build DVE op
HALF = 32768  # int16 index range
REP = 64  # h2 replication (64x2 bf16 cols -> 256B rows)
GATHER_BF16 = True  # layer-1 gather table + chunk matmuls in bf16


# --------------------------------------------------------------------------
# Host preprocessing
# --------------------------------------------------------------------------
def _preprocess(x, edge_index, n_cores):
    N = x.shape[0]
    src = np.concatenate(
        [np.asarray(edge_index[0], dtype=np.int64), np.arange(N, dtype=np.int64)]
    )
    dst = np.concatenate(
        [np.asarray(edge_index[1], dtype=np.int64), np.arange(N, dtype=np.int64)]
    )
    deg = np.bincount(dst, minlength=N).astype(np.float64)
    dinv = np.where(deg > 0, 1.0 / np.sqrt(deg), 0.0).astype(np.float32)

    n_local = (N + n_cores - 1) // n_cores
    w_cnt = (n_local + WINDOW - 1) // WINDOW

    order = np.argsort(dst, kind="stable")
    s_src = src[order]
    s_dst = dst[order]

    # table rows: 0 = zero, 1..N = nodes, N+1 = zero.  row(n) = n+1
    # LOW view = rows [0, min(HALF, N+2));  HIGH view = rows [HB, HB+HALF)
    HB = max(0, N + 2 - HALF)
    lowmax_row = min(HALF, N + 2)  # rows < this go to LOW chunks
    pad_low = 0  # zero row 0
    pad_high = N + 1 - HB  # zero row N+1 relative to HB

    # per (core, window): split edges into LOW (row < lowmax) and HIGH
    parts = {}  # (c, w, hi) -> (rows_arr, dstrel_arr)
    counts = np.zeros((2, n_cores, w_cnt), dtype=np.int64)
    for c in range(n_cores):
        base = c * n_local
        for w in range(w_cnt):
            wlo = base + w * WINDOW
            whi = min(base + (w + 1) * WINDOW, base + n_local, N)
            lo_i = np.searchsorted(s_dst, wlo, side="left")
            hi_i = np.searchsorted(s_dst, whi, side="left")
            rows = (s_src[lo_i:hi_i] + 1).astype(np.int64)
            rel = (s_dst[lo_i:hi_i] - wlo).astype(np.float32)
            is_lo = rows < lowmax_row
            parts[(c, w, 0)] = (rows[is_lo], rel[is_lo])
            parts[(c, w, 1)] = (rows[~is_lo] - HB, rel[~is_lo])
            counts[0, c, w] = is_lo.sum()
            counts[1, c, w] = (~is_lo).sum()

    # uniform per-window chunk counts across cores, per section
    kw_lo = np.maximum(1, np.ceil(counts[0] / CHUNK).astype(np.int64).max(axis=0))
    kw_hi = np.maximum(1, np.ceil(counts[1] / CHUNK).astype(np.int64).max(axis=0))
    T_lo, T_hi = int(kw_lo.sum()), int(kw_hi.sum())
    T = T_lo + T_hi

    # chunk order: LOW section (windows in order), then HIGH section
    chunk_win = []  # (window, first_in_sec, last_in_sec, section)
    for sec, kws in ((0, kw_lo), (1, kw_hi)):
        for w in range(w_cnt):
            for k in range(kws[w]):
                chunk_win.append((w, k == 0, k == kws[w] - 1, sec))

    per_core = []
    for c in range(n_cores):
        idx_lin = np.zeros(T * CHUNK, dtype=np.int32)
        dstrel = np.zeros((CHUNK, T), dtype=np.float32)
        t = 0
        for sec, kws, padrow in ((0, kw_lo, pad_low), (1, kw_hi, pad_high)):
            for w in range(w_cnt):
                rows, rel = parts[(c, w, sec)]
                n_e = len(rows)
                n_slots = int(kws[w]) * CHUNK
                buf = np.full(n_slots, padrow, dtype=np.int32)
                buf[:n_e] = rows
                idx_lin[t * CHUNK : t * CHUNK + n_slots] = buf
                rbuf = np.zeros(n_slots, dtype=np.float32)
                rbuf[:n_e] = rel
                dstrel[:, t : t + int(kws[w])] = rbuf.reshape(int(kws[w]), CHUNK).T
                t += int(kws[w])
        assert t == T
        # dma_gather idx layout: [128, T*8] int16; linear i = s*16 + r
        # (rows 0..15, replicated to all 128 partitions)
        idx16 = idx_lin.astype(np.int16).reshape(T * CHUNK // 16, 16).T  # [16, S]
        idx16 = np.tile(idx16, (8, 1))  # [128, S]

        dinvw = np.zeros((WINDOW, w_cnt), dtype=np.float32)
        base = c * n_local
        for w in range(w_cnt):
            wlo = base + w * WINDOW
            whi = min(wlo + WINDOW, base + n_local, N)
            if whi > wlo:
                dinvw[: whi - wlo, w] = dinv[wlo:whi]
        per_core.append({"idx16": idx16, "dstrel": dstrel, "dinvw": dinvw})

    return {
        "n_local": n_local,
        "w_cnt": w_cnt,
        "kw_lo": kw_lo,
        "kw_hi": kw_hi,
        "T_lo": T_lo,
        "T_hi": T_hi,
        "T": T,
        "HB": HB,
        "chunk_win": chunk_win,
        "dinv": dinv,
        "per_core": per_core,
    }


# --------------------------------------------------------------------------
# Device kernel builder (one program, SPMD across cores)
# --------------------------------------------------------------------------
def _build(nc, *, N, n_local, d_in, d_hid, n_cls, pp, n_cores, dt_gat):
    Relu = mybir.ActivationFunctionType.Relu
    Copy = mybir.ActivationFunctionType.Copy
    T, T_lo = pp["T"], pp["T_lo"]
    w_cnt, HB = pp["w_cnt"], pp["HB"]
    chunk_win = pp["chunk_win"]
    d_rep = REP * n_cls  # 64 cols of f32 -> 256B rows

    xtab = nc.dram_tensor("xtab", [N + 2, d_in], dt_gat, kind="ExternalInput")
    w1 = nc.dram_tensor("w1", [d_in, d_hid], F32, kind="ExternalInput")
    w2 = nc.dram_tensor("w2", [d_hid, n_cls], F32, kind="ExternalInput")
    b1bc = nc.dram_tensor("b1bc", [WINDOW, d_hid], F32, kind="ExternalInput")
    b2bc = nc.dram_tensor("b2bc", [WINDOW, n_cls], F32, kind="ExternalInput")
    iota = nc.dram_tensor("iota", [CHUNK, SBATCH * WINDOW], F32, kind="ExternalInput")
    ident = nc.dram_tensor("ident", [WINDOW, WINDOW], F32, kind="ExternalInput")
    idx_t = nc.dram_tensor("idx16", [CHUNK, T * 8], I16, kind="ExternalInput")
    dstrel_t = nc.dram_tensor("dstrel", [CHUNK, T], F32, kind="ExternalInput")
    dinvw_t = nc.dram_tensor("dinvw", [WINDOW, w_cnt], F32, kind="ExternalInput")
    out_t = nc.dram_tensor("out", [n_local, n_cls], F32, kind="ExternalOutput")

    h2loc = nc.dram_tensor("h2loc", [n_local, d_rep], BF16)
    h2tab = nc.dram_tensor("h2tab", [N + 2, d_rep], BF16, addr_space="Shared")

    # per-section gather groups: (sec, t0, n, queue)
    groups = []
    qn = 0
    for sec, tlo, thi in ((0, 0, T_lo), (1, T_lo, T)):
        t0 = tlo
        while t0 < thi:
            n = min(GSZ, thi - t0)
            groups.append((sec, t0, n, qn % 4))
            qn += 1
            t0 += n

    def tab_view(tab):
        return [
            tab[0 : min(HALF, N + 2), :],
            tab[HB : min(HB + HALF, N + 2), :],
        ]

    with tile.TileContext(nc) as tc:
        with (
            tc.tile_pool(name="const", bufs=1) as cpool,
            tc.tile_pool(name="gbuf", bufs=3) as gpool,
            tc.tile_pool(name="g2buf", bufs=3) as g2pool,
            tc.tile_pool(name="sbat", bufs=3) as spool,
            tc.tile_pool(name="sbat2", bufs=3) as s2pool,
            tc.tile_pool(name="wtmp", bufs=3) as wpool,
            tc.tile_pool(name="aggs", bufs=1) as apool,
            tc.tile_pool(name="psA", bufs=3, space="PSUM") as psA,
            tc.tile_pool(name="psW", bufs=3, space="PSUM") as psW,
        ):
            # ---- constants into SBUF ----
            w1_sb = cpool.tile([d_in, d_hid], F32, tag="w1")
            nc.sync.dma_start(out=w1_sb[:], in_=w1[:])
            w2_sb = cpool.tile([d_hid, n_cls], F32, tag="w2")
            nc.sync.dma_start(out=w2_sb[:], in_=w2[:])
            b1_sb = cpool.tile([WINDOW, d_hid], F32, tag="b1")
            nc.sync.dma_start(out=b1_sb[:], in_=b1bc[:])
            b2_sb = cpool.tile([WINDOW, n_cls], F32, tag="b2")
            nc.sync.dma_start(out=b2_sb[:], in_=b2bc[:])
            iota_sb = cpool.tile([CHUNK, SBATCH * WINDOW], F32, tag="iota")
            nc.sync.dma_start(out=iota_sb[:], in_=iota[:])
            id_sb = cpool.tile([WINDOW, WINDOW], F32, tag="ident")
            nc.sync.dma_start(out=id_sb[:], in_=ident[:])
            idx_sb = cpool.tile([CHUNK, T * 8], I16, tag="idx")
            nc.sync.dma_start(out=idx_sb[:], in_=idx_t[:])
            dstrel_sb = cpool.tile([CHUNK, T], F32, tag="dstrel")
            nc.sync.dma_start(out=dstrel_sb[:], in_=dstrel_t[:])
            dinvw_sb = cpool.tile([WINDOW, w_cnt], F32, tag="dinvw")
            nc.sync.dma_start(out=dinvw_sb[:], in_=dinvw_t[:])

            zrow = cpool.tile([1, d_rep], BF16, tag="zrow")
            nc.vector.memset(zrow[:], 0.0)
            nc.sync.dma_start(out=h2tab[0:1, :], in_=zrow[:1, :])
            nc.sync.dma_start(out=h2tab[N + 1 : N + 2, :], in_=zrow[:1, :])

            def build_s(pool, t0, n, nm):
                """one-hot S for chunks [t0, t0+n) in one DVE op."""
                s_tile = pool.tile([CHUNK, SBATCH * WINDOW], BF16, tag="s", name=nm)
                rel_b = (
                    dstrel_sb[:, t0 : t0 + n]
                    .rearrange("p (b one) -> p b one", one=1)
                    .to_broadcast([CHUNK, n, WINDOW])
                )
                io_v = iota_sb[:, : n * WINDOW].rearrange("p (b j) -> p b j", j=WINDOW)
                s_v = s_tile[:, : n * WINDOW].rearrange("p (b j) -> p b j", j=WINDOW)
                nc.vector.tensor_tensor(
                    out=s_v, in0=io_v, in1=rel_b, op=mybir.AluOpType.is_equal
                )
                return s_tile

            # per-window accumulators in SBUF (LOW evicts, HIGH adds on top)
            aggT_sb = apool.tile([d_in, w_cnt * WINDOW], F32, tag="aggT")
            out2_sb = apool.tile([WINDOW, w_cnt * n_cls], F32, tag="out2")

            # =========================== PHASE A ===========================
            psum_of_win = {}
            for sec, t0, n, qn in groups:
                gb = gpool.tile([CHUNK, GSZ, d_in], dt_gat, tag="g", name="gb")
                nc.gpsimd.dma_gather(
                    gb[:, :n, :],
                    tab_view(xtab)[sec],
                    idx_sb[:, t0 * 8 : (t0 + n) * 8],
                    n * CHUNK,
                    n * CHUNK,
                    d_in,
                    single_packet=True,
                    queue_num=qn,
                )
                for bt0 in range(t0, t0 + n, SBATCH):
                    bn = min(SBATCH, t0 + n - bt0)
                    s_tile = build_s(spool, bt0, bn, "sA")
                    for t in range(bt0, bt0 + bn):
                        j = t - bt0
                        w, first, last, _sec = chunk_win[t]
                        if first:
                            psum_of_win[w] = psA.tile(
                                [d_in, WINDOW], F32, tag="agg", name="aggps"
                            )
                        nc.tensor.matmul(
                            out=psum_of_win[w][:],
                            lhsT=gb[:, t - t0, :],
                            rhs=s_tile[:, j * WINDOW : (j + 1) * WINDOW],
                            start=first,
                            stop=last,
                        )
                        if not last:
                            continue
                        ps = psum_of_win.pop(w)
                        wsl = aggT_sb[:, w * WINDOW : (w + 1) * WINDOW]
                        if _sec == 0:
                            nc.scalar.activation(out=wsl, in_=ps[:], func=Copy)
                        else:
                            nc.vector.tensor_tensor(
                                out=wsl, in0=ps[:], in1=wsl, op=mybir.AluOpType.add
                            )
                            _window_epilogue_A(
                                nc, w, wsl, wpool, psW, w1_sb, w2_sb, b1_sb,
                                dinvw_sb, id_sb, h2loc, n_local, d_in, d_hid,
                                n_cls, d_rep,
                            )

            # ======================= h2 exchange ==========================
            if n_cores > 1:
                nc.gpsimd.collective_compute(
                    "AllGather",
                    mybir.AluOpType.bypass,
                    replica_groups=[list(range(n_cores))],
                    ins=[h2loc[:]],
                    outs=[h2tab[1 : 1 + n_cores * n_local, :]],
                )
            else:
                nc.sync.dma_start(out=h2tab[1 : 1 + n_local, :], in_=h2loc[:])

            # =========================== PHASE B ===========================
            psum_of_win = {}
            for sec, t0, n, qn in groups:
                g2 = g2pool.tile([CHUNK, GSZ, d_rep], BF16, tag="g2", name="g2b")
                nc.gpsimd.dma_gather(
                    g2[:, :n, :],
                    tab_view(h2tab)[sec],
                    idx_sb[:, t0 * 8 : (t0 + n) * 8],
                    n * CHUNK,
                    n * CHUNK,
                    d_rep,
                    single_packet=True,
                    queue_num=qn,
                )
                for bt0 in range(t0, t0 + n, SBATCH):
                    bn = min(SBATCH, t0 + n - bt0)
                    s_tile = build_s(s2pool, bt0, bn, "sB")
                    for t in range(bt0, bt0 + bn):
                        j = t - bt0
                        w, first, last, _sec = chunk_win[t]
                        if first:
                            psum_of_win[w] = psA.tile(
                                [WINDOW, n_cls], F32, tag="agg", name="agg2ps"
                            )
                        nc.tensor.matmul(
                            out=psum_of_win[w][:],
                            lhsT=s_tile[:, j * WINDOW : (j + 1) * WINDOW],
                            rhs=g2[:, t - t0, :n_cls],
                            start=first,
                            stop=last,
                        )
                        if not last:
                            continue
                        ps = psum_of_win.pop(w)
                        osl = out2_sb[:, w * n_cls : (w + 1) * n_cls]
                        if _sec == 0:
                            nc.scalar.activation(out=osl, in_=ps[:], func=Copy)
                        else:
                            ob = wpool.tile([WINDOW, n_cls], F32, tag="ob")
                            nc.vector.tensor_tensor(
                                out=ob[:], in0=ps[:], in1=osl, op=mybir.AluOpType.add
                            )
                            ob2 = wpool.tile([WINDOW, n_cls], F32, tag="ob2")
                            nc.vector.tensor_scalar(
                                out=ob2[:],
                                in0=ob[:],
                                scalar1=dinvw_sb[:, w : w + 1],
                                scalar2=None,
                                op0=mybir.AluOpType.mult,
                            )
                            ob3 = wpool.tile([WINDOW, n_cls], F32, tag="ob3")
                            nc.vector.tensor_tensor(
                                out=ob3[:], in0=ob2[:], in1=b2_sb[:],
                                op=mybir.AluOpType.add,
                            )
                            nrows = min(WINDOW, n_local - w * WINDOW)
                            nc.sync.dma_start(
                                out=out_t[w * WINDOW : w * WINDOW + nrows, :],
                                in_=ob3[:nrows, :],
                            )

    nc.compile()
    return nc


def _window_epilogue_A(
    nc, w, aggT, wpool, psW, w1_sb, w2_sb, b1_sb, dinvw_sb, id_sb,
    h2loc, n_local, d_in, d_hid, n_cls, d_rep,
):
    """aggT [d_in, WINDOW] in SBUF -> replicated h2 rows in DRAM."""
    Relu = mybir.ActivationFunctionType.Relu
    Copy = mybir.ActivationFunctionType.Copy

    # h1 [dst, hid] = aggT.T @ W1
    h1_ps = psW.tile([WINDOW, d_hid], F32, tag="wps", name="h1_ps")
    nc.tensor.matmul(out=h1_ps[:], lhsT=aggT, rhs=w1_sb[:], start=True, stop=True)
    # scale by dinv[dst] (per-partition), + b1, relu
    r_sb = wpool.tile([WINDOW, d_hid], F32, tag="r")
    nc.vector.tensor_scalar(
        out=r_sb[:],
        in0=h1_ps[:],
        scalar1=dinvw_sb[:, w : w + 1],
        scalar2=None,
        op0=mybir.AluOpType.mult,
    )
    r2_sb = wpool.tile([WINDOW, d_hid], F32, tag="r2")
    nc.vector.tensor_tensor(
        out=r2_sb[:], in0=r_sb[:], in1=b1_sb[:], op=mybir.AluOpType.add
    )
    r3_sb = wpool.tile([WINDOW, d_hid], F32, tag="r3")
    nc.scalar.activation(out=r3_sb[:], in_=r2_sb[:], func=Relu)
    # transpose -> [hid, dst]
    rT_ps = psW.tile([d_hid, WINDOW], F32, tag="wps", name="rT_ps")
    nc.tensor.transpose(out=rT_ps[:], in_=r3_sb[:], identity=id_sb[:])
    rT_sb = wpool.tile([d_hid, WINDOW], F32, tag="rTs")
    nc.scalar.activation(out=rT_sb[:], in_=rT_ps[:], func=Copy)
    # h2 [dst, n_cls] = rT.T @ W2; scale by dinv[dst]; replicate REP x
    h2_ps = psW.tile([WINDOW, n_cls], F32, tag="wps", name="h2_ps")
    nc.tensor.matmul(out=h2_ps[:], lhsT=rT_sb[:], rhs=w2_sb[:], start=True, stop=True)
    h2_sb = wpool.tile([WINDOW, d_rep], BF16, tag="h2s")
    nc.vector.tensor_scalar(
        out=h2_sb[:].rearrange("p (r c) -> p r c", c=n_cls),
        in0=h2_ps[:]
        .rearrange("p (one c) -> p one c", one=1)
        .to_broadcast([WINDOW, REP, n_cls]),
        scalar1=dinvw_sb[:, w : w + 1],
        scalar2=None,
        op0=mybir.AluOpType.mult,
    )
    nrows = min(WINDOW, n_local - w * WINDOW)
    nc.sync.dma_start(
        out=h2loc[w * WINDOW : w * WINDOW + nrows, :], in_=h2_sb[:nrows, :]
    )


# --------------------------------------------------------------------------
# Entry point
# --------------------------------------------------------------------------
def _make_inputs(x, W1, b1, W2, b2, pp, dt_np):
    N, d_in = x.shape
    W1 = np.asarray(W1, np.float32)
    b1 = np.asarray(b1, np.float32)
    W2 = np.asarray(W2, np.float32)
    b2 = np.asarray(b2, np.float32)
    d_hid = W1.shape[1]
    n_cls = W2.shape[1]
    xtab = np.concatenate(
        [
            np.zeros((1, d_in), np.float32),
            x * pp["dinv"][:, None],
            np.zeros((1, d_in), np.float32),
        ]
    ).astype(dt_np)
    iota_arr = np.broadcast_to(
        np.tile(np.arange(WINDOW, dtype=np.float32), SBATCH),
        (CHUNK, SBATCH * WINDOW),
    ).copy()
    shared = {
        "xtab": xtab,
        "w1": W1,
        "w2": W2,
        "b1bc": np.broadcast_to(b1, (WINDOW, d_hid)).astype(np.float32).copy(),
        "b2bc": np.broadcast_to(b2, (WINDOW, n_cls)).astype(np.float32).copy(),
        "iota": iota_arr,
        "ident": np.eye(WINDOW, dtype=np.float32),
    }
    in_maps = []
    for pc in pp["per_core"]:
        m = dict(shared)
        m["idx16"] = pc["idx16"]
        m["dstrel"] = pc["dstrel"]
        m["dinvw"] = pc["dinvw"]
        in_maps.append(m)
    return in_maps


def _run(x, edge_index, W1, b1, W2, b2, n_cores, trace=False):
    x = np.asarray(x, dtype=np.float32)
    N, d_in = x.shape
    d_hid = np.asarray(W1).shape[1]
    n_cls = np.asarray(W2).shape[1]
    assert d_in == 128 and d_hid == 128

    pp = _preprocess(x, edge_index, n_cores)
    dt_gat = BF16 if GATHER_BF16 else F32
    np_gat = np.dtype("bfloat16") if GATHER_BF16 else np.dtype("float32")

    nc = bacc.Bacc("TRN2", target_bir_lowering=False, debug=False, num_swdge_queues=4)
    _build(
        nc,
        N=N,
        n_local=pp["n_local"],
        d_in=d_in,
        d_hid=d_hid,
        n_cls=n_cls,
        pp=pp,
        n_cores=n_cores,
        dt_gat=dt_gat,
    )

    import ml_dtypes  # noqa

    in_maps = _make_inputs(x, W1, b1, W2, b2, pp, np_gat)
    res = run_bass_kernel_spmd(nc, in_maps, list(range(n_cores)), trace=trace)
    outs = [res.results[c]["out"] for c in range(n_cores)]
    full = np.concatenate(outs, axis=0)[:N]
    return full.astype(np.float32), res


def kernel(x, edge_index, W1, b1, W2, b2):
    out, _ = _run(x, edge_index, W1, b1, W2, b2, N_CORES)
    return out



# revision 4
# speedup vs baseline: 1.7511x; 1.3800x over previous
"""GCN 2-layer (PyG GCNConv x2 + ReLU) Bass kernel for Trainium2, 8-core SPMD.

Strategy:
  - Host: add self-loops, compute symmetric normalization dinv = deg^-1/2,
    fold dinv[src] into a prescaled gather table (x * dinv), shard dst nodes
    contiguously across 8 cores, sort each core's edges by dst into 128-node
    "windows", pack edges into 128-edge "chunks" (one matmul each).
    dma_gather uses int16 indices, so the node table is addressed via two
    32768-row views (LOW/HIGH); each window's edges are split into LOW chunks
    and HIGH chunks, and the kernel runs all LOW chunks (accumulating per
    window in PSUM, evicting to SBUF), then all HIGH chunks (added on top).
  - Device per core:
      Phase A (layer 1): dma_gather source rows of the prescaled x-table ->
        G [128e, d_in]; build one-hot S [128e, 128dst] on DVE (iota ==
        dst_rel); PE matmul accumulates G.T @ S into PSUM [d_in, 128dst]
        per window (aggregated x per dst, transposed).  Per window: x W1
        (PE), scale by dinv[dst], +b1, ReLU; transpose (PE); x W2; scale by
        dinv[dst]; replicate 32x -> 256B rows of the h2 table, DMA out.
      AllGather h2 shards -> full [N, 64] table.
      Phase B (layer 2): same chunk structure; gather h2 rows, matmul
        S.T @ G2[:, :2] accumulated per window; scale by dinv[dst], +b2.
"""

import numpy as np

import concourse.bass as bass
import concourse.mybir as mybir
import concourse.tile as tile
from concourse import bacc
from concourse.bass_utils import run_bass_kernel_spmd

F32 = mybir.dt.float32
BF16 = mybir.dt.bfloat16
I16 = mybir.dt.int16

N_CORES = 8
WINDOW = 128  # dst nodes per PSUM accumulation window
CHUNK = 128  # edges per matmul chunk
GSZ = 8  # max chunks per dma_gather instruction (1024 idxs, single-packet)
SBATCH = 8  # chunks per S-build DVE op
HALF = 32768  # int16 index range
REP = 64  # h2 replication (64x2 bf16 cols -> 256B rows)
GATHER_BF16 = True  # layer-1 gather table + chunk matmuls in bf16


# --------------------------------------------------------------------------
# Host preprocessing
# --------------------------------------------------------------------------
def _preprocess(x, edge_index, n_cores):
    N = x.shape[0]
    src = np.concatenate(
        [np.asarray(edge_index[0], dtype=np.int64), np.arange(N, dtype=np.int64)]
    )
    dst = np.concatenate(
        [np.asarray(edge_index[1], dtype=np.int64), np.arange(N, dtype=np.int64)]
    )
    deg = np.bincount(dst, minlength=N).astype(np.float64)
    dinv = np.where(deg > 0, 1.0 / np.sqrt(deg), 0.0).astype(np.float32)

    n_local = (N + n_cores - 1) // n_cores
    w_cnt = (n_local + WINDOW - 1) // WINDOW

    order = np.argsort(dst, kind="stable")
    s_src = src[order]
    s_dst = dst[order]

    # table rows: 0 = zero, 1..N = nodes, N+1 = zero.  row(n) = n+1
    # LOW view = rows [0, min(HALF, N+2));  HIGH view = rows [HB, HB+HALF)
    HB = max(0, N + 2 - HALF)
    lowmax_row = min(HALF, N + 2)  # rows < this go to LOW chunks
    pad_low = 0  # zero row 0
    pad_high = N + 1 - HB  # zero row N+1 relative to HB

    # per (core, window): split edges into LOW (row < lowmax) and HIGH
    parts = {}  # (c, w, hi) -> (rows_arr, dstrel_arr)
    counts = np.zeros((2, n_cores, w_cnt), dtype=np.int64)
    for c in range(n_cores):
        base = c * n_local
        for w in range(w_cnt):
            wlo = base + w * WINDOW
            whi = min(base + (w + 1) * WINDOW, base + n_local, N)
            lo_i = np.searchsorted(s_dst, wlo, side="left")
            hi_i = np.searchsorted(s_dst, whi, side="left")
            rows = (s_src[lo_i:hi_i] + 1).astype(np.int64)
            rel = (s_dst[lo_i:hi_i] - wlo).astype(np.float32)
            is_lo = rows < lowmax_row
            parts[(c, w, 0)] = (rows[is_lo], rel[is_lo])
            parts[(c, w, 1)] = (rows[~is_lo] - HB, rel[~is_lo])
            counts[0, c, w] = is_lo.sum()
            counts[1, c, w] = (~is_lo).sum()

    # uniform per-window chunk counts across cores, per section
    kw_lo = np.maximum(1, np.ceil(counts[0] / CHUNK).astype(np.int64).max(axis=0))
    kw_hi = np.maximum(1, np.ceil(counts[1] / CHUNK).astype(np.int64).max(axis=0))
    T_lo, T_hi = int(kw_lo.sum()), int(kw_hi.sum())
    T = T_lo + T_hi

    # chunk order: LOW section (windows in order), then HIGH section
    chunk_win = []  # (window, first_in_sec, last_in_sec, section)
    for sec, kws in ((0, kw_lo), (1, kw_hi)):
        for w in range(w_cnt):
            for k in range(kws[w]):
                chunk_win.append((w, k == 0, k == kws[w] - 1, sec))

    per_core = []
    for c in range(n_cores):
        idx_lin = np.zeros(T * CHUNK, dtype=np.int32)
        dstrel = np.zeros((CHUNK, T), dtype=np.float32)
        t = 0
        for sec, kws, padrow in ((0, kw_lo, pad_low), (1, kw_hi, pad_high)):
            for w in range(w_cnt):
                rows, rel = parts[(c, w, sec)]
                n_e = len(rows)
                n_slots = int(kws[w]) * CHUNK
                buf = np.full(n_slots, padrow, dtype=np.int32)
                buf[:n_e] = rows
                idx_lin[t * CHUNK : t * CHUNK + n_slots] = buf
                rbuf = np.zeros(n_slots, dtype=np.float32)
                rbuf[:n_e] = rel
                dstrel[:, t : t + int(kws[w])] = rbuf.reshape(int(kws[w]), CHUNK).T
                t += int(kws[w])
        assert t == T
        # dma_gather idx layout: [128, T*8] int16; linear i = s*16 + r
        # (rows 0..15, replicated to all 128 partitions)
        idx16 = idx_lin.astype(np.int16).reshape(T * CHUNK // 16, 16).T  # [16, S]
        idx16 = np.tile(idx16, (8, 1))  # [128, S]

        dinvw = np.zeros((WINDOW, w_cnt), dtype=np.float32)
        base = c * n_local
        for w in range(w_cnt):
            wlo = base + w * WINDOW
            whi = min(wlo + WINDOW, base + n_local, N)
            if whi > wlo:
                dinvw[: whi - wlo, w] = dinv[wlo:whi]
        per_core.append({"idx16": idx16, "dstrel": dstrel, "dinvw": dinvw})

    return {
        "n_local": n_local,
        "w_cnt": w_cnt,
        "kw_lo": kw_lo,
        "kw_hi": kw_hi,
        "T_lo": T_lo,
        "T_hi": T_hi,
        "T": T,
        "HB": HB,
        "chunk_win": chunk_win,
        "dinv": dinv,
        "per_core": per_core,
    }


# --------------------------------------------------------------------------
# Device kernel builder (one program, SPMD across cores)
# --------------------------------------------------------------------------
def _build(nc, *, N, n_local, d_in, d_hid, n_cls, pp, n_cores, dt_gat):
    Relu = mybir.ActivationFunctionType.Relu
    Copy = mybir.ActivationFunctionType.Copy
    T, T_lo = pp["T"], pp["T_lo"]
    w_cnt, HB = pp["w_cnt"], pp["HB"]
    chunk_win = pp["chunk_win"]
    d_rep = REP * n_cls  # 64 cols of f32 -> 256B rows

    xtab = nc.dram_tensor("xtab", [N + 2, d_in], dt_gat, kind="ExternalInput")
    w1 = nc.dram_tensor("w1", [d_in, d_hid], F32, kind="ExternalInput")
    w2 = nc.dram_tensor("w2", [d_hid, n_cls], F32, kind="ExternalInput")
    b1bc = nc.dram_tensor("b1bc", [WINDOW, d_hid], F32, kind="ExternalInput")
    b2bc = nc.dram_tensor("b2bc", [WINDOW, n_cls], F32, kind="ExternalInput")
    iota = nc.dram_tensor("iota", [CHUNK, SBATCH * WINDOW], F32, kind="ExternalInput")
    ident = nc.dram_tensor("ident", [WINDOW, WINDOW], F32, kind="ExternalInput")
    idx_t = nc.dram_tensor("idx16", [CHUNK, T * 8], I16, kind="ExternalInput")
    dstrel_t = nc.dram_tensor("dstrel", [CHUNK, T], F32, kind="ExternalInput")
    dinvw_t = nc.dram_tensor("dinvw", [WINDOW, w_cnt], F32, kind="ExternalInput")
    out_t = nc.dram_tensor("out", [n_local, n_cls], F32, kind="ExternalOutput")

    h2loc = nc.dram_tensor("h2loc", [n_local, d_rep], BF16)
    h2tab = nc.dram_tensor("h2tab", [N + 2, d_rep], BF16, addr_space="Shared")

    # per-section gather groups: (sec, t0, n, queue)
    groups = []
    qn = 0
    for sec, tlo, thi in ((0, 0, T_lo), (1, T_lo, T)):
        t0 = tlo
        while t0 < thi:
            n = min(GSZ, thi - t0)
            groups.append((sec, t0, n, qn % 4))
            qn += 1
            t0 += n

    def tab_view(tab):
        return [
            tab[0 : min(HALF, N + 2), :],
            tab[HB : min(HB + HALF, N + 2), :],
        ]

    with tile.TileContext(nc) as tc:
        with (
            tc.tile_pool(name="const", bufs=1) as cpool,
            tc.tile_pool(name="gbuf", bufs=8) as gpool,
            tc.tile_pool(name="g2buf", bufs=8) as g2pool,
            tc.tile_pool(name="sbat", bufs=6) as spool,
            tc.tile_pool(name="sbat2", bufs=6) as s2pool,
            tc.tile_pool(name="wtmp", bufs=3) as wpool,
            tc.tile_pool(name="aggs", bufs=1) as apool,
            tc.tile_pool(name="psA", bufs=4, space="PSUM") as psA,
            tc.tile_pool(name="psW", bufs=3, space="PSUM") as psW,
        ):
            # ---- constants into SBUF ----
            w1_sb = cpool.tile([d_in, d_hid], F32, tag="w1")
            nc.sync.dma_start(out=w1_sb[:], in_=w1[:])
            w2_sb = cpool.tile([d_hid, n_cls], F32, tag="w2")
            nc.sync.dma_start(out=w2_sb[:], in_=w2[:])
            b1_sb = cpool.tile([WINDOW, d_hid], F32, tag="b1")
            nc.sync.dma_start(out=b1_sb[:], in_=b1bc[:])
            b2_sb = cpool.tile([WINDOW, n_cls], F32, tag="b2")
            nc.sync.dma_start(out=b2_sb[:], in_=b2bc[:])
            iota_sb = cpool.tile([CHUNK, SBATCH * WINDOW], F32, tag="iota")
            nc.sync.dma_start(out=iota_sb[:], in_=iota[:])
            id_sb = cpool.tile([WINDOW, WINDOW], F32, tag="ident")
            nc.sync.dma_start(out=id_sb[:], in_=ident[:])
            idx_sb = cpool.tile([CHUNK, T * 8], I16, tag="idx")
            nc.sync.dma_start(out=idx_sb[:], in_=idx_t[:])
            dstrel_sb = cpool.tile([CHUNK, T], F32, tag="dstrel")
            nc.sync.dma_start(out=dstrel_sb[:], in_=dstrel_t[:])
            dinvw_sb = cpool.tile([WINDOW, w_cnt], F32, tag="dinvw")
            nc.sync.dma_start(out=dinvw_sb[:], in_=dinvw_t[:])

            zrow = cpool.tile([1, d_rep], BF16, tag="zrow")
            nc.vector.memset(zrow[:], 0.0)
            nc.sync.dma_start(out=h2tab[0:1, :], in_=zrow[:1, :])
            nc.sync.dma_start(out=h2tab[N + 1 : N + 2, :], in_=zrow[:1, :])

            def build_s(pool, t0, n, nm):
                """one-hot S for chunks [t0, t0+n) in one DVE op."""
                s_tile = pool.tile([CHUNK, SBATCH * WINDOW], BF16, tag="s", name=nm)
                rel_b = (
                    dstrel_sb[:, t0 : t0 + n]
                    .rearrange("p (b one) -> p b one", one=1)
                    .to_broadcast([CHUNK, n, WINDOW])
                )
                io_v = iota_sb[:, : n * WINDOW].rearrange("p (b j) -> p b j", j=WINDOW)
                s_v = s_tile[:, : n * WINDOW].rearrange("p (b j) -> p b j", j=WINDOW)
                nc.vector.tensor_tensor(
                    out=s_v, in0=io_v, in1=rel_b, op=mybir.AluOpType.is_equal
                )
                return s_tile

            # per-window accumulators in SBUF (LOW evicts, HIGH adds on top)
            aggT_sb = apool.tile([d_in, w_cnt * WINDOW], F32, tag="aggT")
            out2_sb = apool.tile([WINDOW, w_cnt * n_cls], F32, tag="out2")

            # =========================== PHASE A ===========================
            psum_of_win = {}
            for sec, t0, n, qn in groups:
                gb = gpool.tile([CHUNK, GSZ, d_in], dt_gat, tag="g", name="gb")
                nc.gpsimd.dma_gather(
                    gb[:, :n, :],
                    tab_view(xtab)[sec],
                    idx_sb[:, t0 * 8 : (t0 + n) * 8],
                    n * CHUNK,
                    n * CHUNK,
                    d_in,
                    single_packet=True,
                    queue_num=qn,
                )
                for bt0 in range(t0, t0 + n, SBATCH):
                    bn = min(SBATCH, t0 + n - bt0)
                    s_tile = build_s(spool, bt0, bn, "sA")
                    for t in range(bt0, bt0 + bn):
                        j = t - bt0
                        w, first, last, _sec = chunk_win[t]
                        if first:
                            psum_of_win[w] = psA.tile(
                                [d_in, WINDOW], F32, tag="agg", name="aggps"
                            )
                        nc.tensor.matmul(
                            out=psum_of_win[w][:],
                            lhsT=gb[:, t - t0, :],
                            rhs=s_tile[:, j * WINDOW : (j + 1) * WINDOW],
                            start=first,
                            stop=last,
                        )
                        if not last:
                            continue
                        ps = psum_of_win.pop(w)
                        wsl = aggT_sb[:, w * WINDOW : (w + 1) * WINDOW]
                        if _sec == 0:
                            nc.scalar.activation(out=wsl, in_=ps[:], func=Copy)
                        else:
                            nc.vector.tensor_tensor(
                                out=wsl, in0=ps[:], in1=wsl, op=mybir.AluOpType.add
                            )
                            _window_epilogue_A(
                                nc, w, wsl, wpool, psW, w1_sb, w2_sb, b1_sb,
                                dinvw_sb, id_sb, h2loc, n_local, d_in, d_hid,
                                n_cls, d_rep,
                            )

            # ======================= h2 exchange ==========================
            if n_cores > 1:
                nc.gpsimd.collective_compute(
                    "AllGather",
                    mybir.AluOpType.bypass,
                    replica_groups=[list(range(n_cores))],
                    ins=[h2loc[:]],
                    outs=[h2tab[1 : 1 + n_cores * n_local, :]],
                )
            else:
                nc.sync.dma_start(out=h2tab[1 : 1 + n_local, :], in_=h2loc[:])

            # =========================== PHASE B ===========================
            psum_of_win = {}
            for sec, t0, n, qn in groups:
                g2 = g2pool.tile([CHUNK, GSZ, d_rep], BF16, tag="g2", name="g2b")
                nc.gpsimd.dma_gather(
                    g2[:, :n, :],
                    tab_view(h2tab)[sec],
                    idx_sb[:, t0 * 8 : (t0 + n) * 8],
                    n * CHUNK,
                    n * CHUNK,
                    d_rep,
                    single_packet=True,
                    queue_num=qn,
                )
                for bt0 in range(t0, t0 + n, SBATCH):
                    bn = min(SBATCH, t0 + n - bt0)
                    s_tile = build_s(s2pool, bt0, bn, "sB")
                    for t in range(bt0, bt0 + bn):
                        j = t - bt0
                        w, first, last, _sec = chunk_win[t]
                        if first:
                            psum_of_win[w] = psA.tile(
                                [WINDOW, n_cls], F32, tag="agg", name="agg2ps"
                            )
                        nc.tensor.matmul(
                            out=psum_of_win[w][:],
                            lhsT=s_tile[:, j * WINDOW : (j + 1) * WINDOW],
                            rhs=g2[:, t - t0, :n_cls],
                            start=first,
                            stop=last,
                        )
                        if not last:
                            continue
                        ps = psum_of_win.pop(w)
                        osl = out2_sb[:, w * n_cls : (w + 1) * n_cls]
                        if _sec == 0:
                            nc.scalar.activation(out=osl, in_=ps[:], func=Copy)
                        else:
                            ob = wpool.tile([WINDOW, n_cls], F32, tag="ob")
                            nc.vector.tensor_tensor(
                                out=ob[:], in0=ps[:], in1=osl, op=mybir.AluOpType.add
                            )
                            ob2 = wpool.tile([WINDOW, n_cls], F32, tag="ob2")
                            nc.vector.tensor_scalar(
                                out=ob2[:],
                                in0=ob[:],
                                scalar1=dinvw_sb[:, w : w + 1],
                                scalar2=None,
                                op0=mybir.AluOpType.mult,
                            )
                            ob3 = wpool.tile([WINDOW, n_cls], F32, tag="ob3")
                            nc.vector.tensor_tensor(
                                out=ob3[:], in0=ob2[:], in1=b2_sb[:],
                                op=mybir.AluOpType.add,
                            )
                            nrows = min(WINDOW, n_local - w * WINDOW)
                            nc.sync.dma_start(
                                out=out_t[w * WINDOW : w * WINDOW + nrows, :],
                                in_=ob3[:nrows, :],
                            )

    nc.compile()
    return nc


def _window_epilogue_A(
    nc, w, aggT, wpool, psW, w1_sb, w2_sb, b1_sb, dinvw_sb, id_sb,
    h2loc, n_local, d_in, d_hid, n_cls, d_rep,
):
    """aggT [d_in, WINDOW] in SBUF -> replicated h2 rows in DRAM."""
    Relu = mybir.ActivationFunctionType.Relu
    Copy = mybir.ActivationFunctionType.Copy

    # h1 [dst, hid] = aggT.T @ W1
    h1_ps = psW.tile([WINDOW, d_hid], F32, tag="wps", name="h1_ps")
    nc.tensor.matmul(out=h1_ps[:], lhsT=aggT, rhs=w1_sb[:], start=True, stop=True)
    # scale by dinv[dst] (per-partition), + b1, relu
    r_sb = wpool.tile([WINDOW, d_hid], F32, tag="r")
    nc.vector.tensor_scalar(
        out=r_sb[:],
        in0=h1_ps[:],
        scalar1=dinvw_sb[:, w : w + 1],
        scalar2=None,
        op0=mybir.AluOpType.mult,
    )
    r2_sb = wpool.tile([WINDOW, d_hid], F32, tag="r2")
    nc.vector.tensor_tensor(
        out=r2_sb[:], in0=r_sb[:], in1=b1_sb[:], op=mybir.AluOpType.add
    )
    r3_sb = wpool.tile([WINDOW, d_hid], F32, tag="r3")
    nc.scalar.activation(out=r3_sb[:], in_=r2_sb[:], func=Relu)
    # transpose -> [hid, dst]
    rT_ps = psW.tile([d_hid, WINDOW], F32, tag="wps", name="rT_ps")
    nc.tensor.transpose(out=rT_ps[:], in_=r3_sb[:], identity=id_sb[:])
    rT_sb = wpool.tile([d_hid, WINDOW], F32, tag="rTs")
    nc.scalar.activation(out=rT_sb[:], in_=rT_ps[:], func=Copy)
    # h2 [dst, n_cls] = rT.T @ W2; scale by dinv[dst]; replicate REP x
    h2_ps = psW.tile([WINDOW, n_cls], F32, tag="wps", name="h2_ps")
    nc.tensor.matmul(out=h2_ps[:], lhsT=rT_sb[:], rhs=w2_sb[:], start=True, stop=True)
    h2_sb = wpool.tile([WINDOW, d_rep], BF16, tag="h2s")
    nc.vector.tensor_scalar(
        out=h2_sb[:].rearrange("p (r c) -> p r c", c=n_cls),
        in0=h2_ps[:]
        .rearrange("p (one c) -> p one c", one=1)
        .to_broadcast([WINDOW, REP, n_cls]),
        scalar1=dinvw_sb[:, w : w + 1],
        scalar2=None,
        op0=mybir.AluOpType.mult,
    )
    nrows = min(WINDOW, n_local - w * WINDOW)
    nc.sync.dma_start(
        out=h2loc[w * WINDOW : w * WINDOW + nrows, :], in_=h2_sb[:nrows, :]
    )


# --------------------------------------------------------------------------
# Entry point
# --------------------------------------------------------------------------
def _make_inputs(x, W1, b1, W2, b2, pp, dt_np):
    N, d_in = x.shape
    W1 = np.asarray(W1, np.float32)
    b1 = np.asarray(b1, np.float32)
    W2 = np.asarray(W2, np.float32)
    b2 = np.asarray(b2, np.float32)
    d_hid = W1.shape[1]
    n_cls = W2.shape[1]
    xtab = np.concatenate(
        [
            np.zeros((1, d_in), np.float32),
            x * pp["dinv"][:, None],
            np.zeros((1, d_in), np.float32),
        ]
    ).astype(dt_np)
    iota_arr = np.broadcast_to(
        np.tile(np.arange(WINDOW, dtype=np.float32), SBATCH),
        (CHUNK, SBATCH * WINDOW),
    ).copy()
    shared = {
        "xtab": xtab,
        "w1": W1,
        "w2": W2,
        "b1bc": np.broadcast_to(b1, (WINDOW, d_hid)).astype(np.float32).copy(),
        "b2bc": np.broadcast_to(b2, (WINDOW, n_cls)).astype(np.float32).copy(),
        "iota": iota_arr,
        "ident": np.eye(WINDOW, dtype=np.float32),
    }
    in_maps = []
    for pc in pp["per_core"]:
        m = dict(shared)
        m["idx16"] = pc["idx16"]
        m["dstrel"] = pc["dstrel"]
        m["dinvw"] = pc["dinvw"]
        in_maps.append(m)
    return in_maps


def _run(x, edge_index, W1, b1, W2, b2, n_cores, trace=False):
    x = np.asarray(x, dtype=np.float32)
    N, d_in = x.shape
    d_hid = np.asarray(W1).shape[1]
    n_cls = np.asarray(W2).shape[1]
    assert d_in == 128 and d_hid == 128

    pp = _preprocess(x, edge_index, n_cores)
    dt_gat = BF16 if GATHER_BF16 else F32
    np_gat = np.dtype("bfloat16") if GATHER_BF16 else np.dtype("float32")

    nc = bacc.Bacc("TRN2", target_bir_lowering=False, debug=False, num_swdge_queues=4)
    _build(
        nc,
        N=N,
        n_local=pp["n_local"],
        d_in=d_in,
        d_hid=d_hid,
        n_cls=n_cls,
        pp=pp,
        n_cores=n_cores,
        dt_gat=dt_gat,
    )

    import ml_dtypes  # noqa

    in_maps = _make_inputs(x, W1, b1, W2, b2, pp, np_gat)
    res = run_bass_kernel_spmd(nc, in_maps, list(range(n_cores)), trace=trace)
    outs = [res.results[c]["out"] for c in range(n_cores)]
    full = np.concatenate(outs, axis=0)[:N]
    return full.astype(np.float32), res


def kernel(x, edge_index, W1, b1, W2, b2):
    out, _ = _run(x, edge_index, W1, b1, W2, b2, N_CORES)
    return out



# revision 7
# speedup vs baseline: 3.0152x; 1.7219x over previous
"""GCN 2-layer (PyG GCNConv x2 + ReLU) Bass kernel for Trainium2, 8-core SPMD.

Strategy:
  - Host: add self-loops, compute symmetric normalization dinv = deg^-1/2,
    fold dinv[src] into a prescaled gather table (x * dinv), shard dst nodes
    contiguously across 8 cores, sort each core's edges by dst into 128-node
    "windows", pack edges into 128-edge "chunks" (one matmul each).
    dma_gather uses int16 indices, so the node table is addressed via two
    32768-row views (LOW/HIGH); each window's edges are split into LOW chunks
    and HIGH chunks, and the kernel runs all LOW chunks (accumulating per
    window in PSUM, evicting to SBUF), then all HIGH chunks (added on top).
  - Device per core:
      Phase A (layer 1): dma_gather source rows of the prescaled x-table ->
        G [128e, d_in]; build one-hot S [128e, 128dst] on DVE (iota ==
        dst_rel); PE matmul accumulates G.T @ S into PSUM [d_in, 128dst]
        per window (aggregated x per dst, transposed).  Per window: x W1
        (PE), scale by dinv[dst], +b1, ReLU; transpose (PE); x W2; scale by
        dinv[dst]; replicate 32x -> 256B rows of the h2 table, DMA out.
      AllGather h2 shards -> full [N, 64] table.
      Phase B (layer 2): same chunk structure; gather h2 rows, matmul
        S.T @ G2[:, :2] accumulated per window; scale by dinv[dst], +b2.
"""

import numpy as np

import concourse.bass as bass
import concourse.mybir as mybir
import concourse.tile as tile
from concourse import bacc
from concourse.bass_utils import run_bass_kernel_spmd

F32 = mybir.dt.float32
BF16 = mybir.dt.bfloat16
I16 = mybir.dt.int16

N_CORES = 8
WINDOW = 128  # dst nodes per PSUM accumulation window
CHUNK = 128  # edges per matmul chunk
GSZ = 8  # max chunks per dma_gather instruction (1024 idxs, single-packet)
SBATCH = 8  # chunks per S-build DVE op
HALF = 32768  # int16 index range
REP = 64  # h2 replication (64x2 bf16 cols -> 256B rows)
GATHER_BF16 = True  # layer-1 gather table + chunk matmuls in bf16


# --------------------------------------------------------------------------
# Host preprocessing
# --------------------------------------------------------------------------
def _preprocess(x, edge_index, n_cores):
    N = x.shape[0]
    src = np.concatenate(
        [np.asarray(edge_index[0], dtype=np.int64), np.arange(N, dtype=np.int64)]
    )
    dst = np.concatenate(
        [np.asarray(edge_index[1], dtype=np.int64), np.arange(N, dtype=np.int64)]
    )
    deg = np.bincount(dst, minlength=N).astype(np.float64)
    dinv = np.where(deg > 0, 1.0 / np.sqrt(deg), 0.0).astype(np.float32)

    # ---- degree-balanced dst binning: 392 bins (8 cores x 49 windows), ----
    # ---- capacity 128 nodes each; LPT greedy on in-degree ----
    import heapq

    w_cnt = 49
    n_bins = n_cores * w_cnt
    n_local = w_cnt * WINDOW  # 6272 (padded shard; empty slots get dinv 0)
    order_nodes = np.argsort(-deg, kind="stable")
    heap = [(0.0, b, 0) for b in range(n_bins)]  # (edge_sum, bin, n_nodes)
    heapq.heapify(heap)
    bin_of = np.empty(N, dtype=np.int64)
    slot_of = np.empty(N, dtype=np.int64)
    stash = []
    for n in order_nodes:
        while True:
            s_, b, cnt = heapq.heappop(heap)
            if cnt < WINDOW:
                break
            stash.append((s_, b, cnt))  # full bin, drop
        bin_of[n] = b
        slot_of[n] = cnt
        heapq.heappush(heap, (s_ + deg[n], b, cnt + 1))
    # permuted position of node n
    permpos = bin_of * WINDOW + slot_of  # in [0, 50176)
    # inverse: node at permuted position p (or -1)
    inv = np.full(n_cores * n_local, -1, dtype=np.int64)
    inv[permpos] = np.arange(N)

    # table rows: 0 = zero, 1..NP = permuted slots, NP+1 = zero.
    # row(node n) = permpos[n] + 1
    # LOW view = rows [0, min(HALF, NP+2)); HIGH view = rows [HB, HB+HALF)
    NP = n_cores * n_local
    HB = max(0, NP + 2 - HALF)
    lowmax_row = min(HALF, NP + 2)
    pad_low = 0
    pad_high = NP + 1 - HB

    rows_all = permpos[src] + 1  # gather row per edge (permuted indexing)
    dstbin = bin_of[dst]
    dstslot = slot_of[dst]

    # group edges by dst bin
    eorder = np.argsort(dstbin, kind="stable")
    g_rows = rows_all[eorder]
    g_slot = dstslot[eorder]
    g_bin = dstbin[eorder]
    starts = np.searchsorted(g_bin, np.arange(n_bins + 1))

    # per (core, window): split edges into LOW/HIGH with flex edges
    # (rows in [HB, lowmax) fit either view). kw must be uniform across
    # cores, so pick per-window section targets jointly: kw_lo = max_c
    # ceil(lo_only/128), all cores fill LOW to that boundary with flex.
    split = {}
    for b in range(n_bins):
        c, w = divmod(b, w_cnt)
        rows = g_rows[starts[b] : starts[b + 1]]
        rel = g_slot[starts[b] : starts[b + 1]].astype(np.float32)
        is_lo_only = rows < HB
        is_hi_only = rows >= lowmax_row
        is_flex = ~is_lo_only & ~is_hi_only
        split[(c, w)] = (
            (rows[is_lo_only], rel[is_lo_only]),
            (rows[is_hi_only], rel[is_hi_only]),
            (rows[is_flex], rel[is_flex]),
        )
    parts = {}
    kw_lo = np.zeros(w_cnt, dtype=np.int64)
    kw_hi = np.zeros(w_cnt, dtype=np.int64)
    for w in range(w_cnt):
        klo_a = max(
            1, max(-(-len(split[(c, w)][0][0]) // CHUNK) for c in range(n_cores))
        )
        khi_a = max(
            -(
                -(len(split[(c, w)][1][0]) + max(
                    0,
                    len(split[(c, w)][2][0])
                    - (klo_a * CHUNK - len(split[(c, w)][0][0])),
                ))
                // CHUNK
            )
            for c in range(n_cores)
        )
        khi_a = max(1, khi_a)
        # alternative: minimize HIGH first
        khi_b = max(
            1, max(-(-len(split[(c, w)][1][0]) // CHUNK) for c in range(n_cores))
        )
        klo_b = max(
            -(
                -(len(split[(c, w)][0][0]) + max(
                    0,
                    len(split[(c, w)][2][0])
                    - (khi_b * CHUNK - len(split[(c, w)][1][0])),
                ))
                // CHUNK
            )
            for c in range(n_cores)
        )
        klo_b = max(1, klo_b)
        if klo_a + khi_a <= klo_b + khi_b:
            kw_lo[w], kw_hi[w], fill_low = klo_a, khi_a, True
        else:
            kw_lo[w], kw_hi[w], fill_low = klo_b, khi_b, False
        for c in range(n_cores):
            (lo_r, lo_s), (hi_r, hi_s), (fx_r, fx_s) = split[(c, w)]
            if fill_low:
                take = min(len(fx_r), kw_lo[w] * CHUNK - len(lo_r))
            else:
                take = len(fx_r) - min(len(fx_r), kw_hi[w] * CHUNK - len(hi_r))
            parts[(c, w, 0)] = (
                np.concatenate([lo_r, fx_r[:take]]),
                np.concatenate([lo_s, fx_s[:take]]),
            )
            parts[(c, w, 1)] = (
                np.concatenate([hi_r, fx_r[take:]]) - HB,
                np.concatenate([hi_s, fx_s[take:]]),
            )
    T_lo, T_hi = int(kw_lo.sum()), int(kw_hi.sum())
    T = T_lo + T_hi

    # chunk order: LOW section (windows in order), then HIGH section
    chunk_win = []  # (window, first_in_sec, last_in_sec, section)
    for sec, kws in ((0, kw_lo), (1, kw_hi)):
        for w in range(w_cnt):
            for k in range(kws[w]):
                chunk_win.append((w, k == 0, k == kws[w] - 1, sec))

    per_core = []
    for c in range(n_cores):
        idx_lin = np.zeros(T * CHUNK, dtype=np.int32)
        dstrel = np.zeros((CHUNK, T), dtype=np.float32)
        t = 0
        for sec, kws, padrow in ((0, kw_lo, pad_low), (1, kw_hi, pad_high)):
            for w in range(w_cnt):
                rows, rel = parts[(c, w, sec)]
                n_e = len(rows)
                n_slots = int(kws[w]) * CHUNK
                buf = np.full(n_slots, padrow, dtype=np.int32)
                buf[:n_e] = rows
                idx_lin[t * CHUNK : t * CHUNK + n_slots] = buf
                rbuf = np.zeros(n_slots, dtype=np.float32)
                rbuf[:n_e] = rel
                dstrel[:, t : t + int(kws[w])] = rbuf.reshape(int(kws[w]), CHUNK).T
                t += int(kws[w])
        assert t == T
        idx16 = idx_lin.astype(np.int16).reshape(T * CHUNK // 16, 16).T  # [16, S]
        idx16 = np.tile(idx16, (8, 1))  # [128, S]

        dinvw = np.zeros((WINDOW, w_cnt), dtype=np.float32)
        for w in range(w_cnt):
            nodes = inv[(c * w_cnt + w) * WINDOW : (c * w_cnt + w + 1) * WINDOW]
            valid = nodes >= 0
            dinvw[valid, w] = dinv[nodes[valid]]
        per_core.append({"idx16": idx16, "dstrel": dstrel, "dinvw": dinvw})

    return {
        "n_local": n_local,
        "w_cnt": w_cnt,
        "kw_lo": kw_lo,
        "kw_hi": kw_hi,
        "T_lo": T_lo,
        "T_hi": T_hi,
        "T": T,
        "HB": HB,
        "chunk_win": chunk_win,
        "dinv": dinv,
        "permpos": permpos,
        "inv": inv,
        "per_core": per_core,
    }


# --------------------------------------------------------------------------
# Device kernel builder (one program, SPMD across cores)
# --------------------------------------------------------------------------
def _build(nc, *, N, n_local, d_in, d_hid, n_cls, pp, n_cores, dt_gat):
    Relu = mybir.ActivationFunctionType.Relu
    Copy = mybir.ActivationFunctionType.Copy
    T, T_lo = pp["T"], pp["T_lo"]
    w_cnt, HB = pp["w_cnt"], pp["HB"]
    chunk_win = pp["chunk_win"]
    d_rep = REP * n_cls  # 64 cols of f32 -> 256B rows

    xtab = nc.dram_tensor("xtab", [N + 2, d_in], dt_gat, kind="ExternalInput")
    w1 = nc.dram_tensor("w1", [d_in, d_hid], F32, kind="ExternalInput")
    w2 = nc.dram_tensor("w2", [d_hid, n_cls], F32, kind="ExternalInput")
    b1bc = nc.dram_tensor("b1bc", [WINDOW, d_hid], F32, kind="ExternalInput")
    b2bc = nc.dram_tensor("b2bc", [WINDOW, n_cls], F32, kind="ExternalInput")
    iota = nc.dram_tensor("iota", [CHUNK, SBATCH * WINDOW], F32, kind="ExternalInput")
    ident = nc.dram_tensor("ident", [WINDOW, WINDOW], F32, kind="ExternalInput")
    idx_t = nc.dram_tensor("idx16", [CHUNK, T * 8], I16, kind="ExternalInput")
    dstrel_t = nc.dram_tensor("dstrel", [CHUNK, T], F32, kind="ExternalInput")
    dinvw_t = nc.dram_tensor("dinvw", [WINDOW, w_cnt], F32, kind="ExternalInput")
    out_t = nc.dram_tensor("out", [n_local, n_cls], F32, kind="ExternalOutput")

    h2loc = nc.dram_tensor("h2loc", [n_local, d_rep], BF16)
    h2tab = nc.dram_tensor("h2tab", [N + 2, d_rep], BF16, addr_space="Shared")

    # per-section gather groups: (sec, t0, n, queue)
    groups = []
    qn = 0
    for sec, tlo, thi in ((0, 0, T_lo), (1, T_lo, T)):
        t0 = tlo
        while t0 < thi:
            n = min(GSZ, thi - t0)
            groups.append((sec, t0, n, qn % 4))
            qn += 1
            t0 += n

    def tab_view(tab):
        return [
            tab[0 : min(HALF, N + 2), :],
            tab[HB : min(HB + HALF, N + 2), :],
        ]

    with tile.TileContext(nc) as tc:
        with (
            tc.tile_pool(name="const", bufs=1) as cpool,
            tc.tile_pool(name="gbuf", bufs=8) as gpool,
            tc.tile_pool(name="g2buf", bufs=8) as g2pool,
            tc.tile_pool(name="sbat", bufs=6) as spool,
            tc.tile_pool(name="sbat2", bufs=6) as s2pool,
            tc.tile_pool(name="wtmp", bufs=3) as wpool,
            tc.tile_pool(name="aggs", bufs=1) as apool,
            tc.tile_pool(name="psA", bufs=4, space="PSUM") as psA,
            tc.tile_pool(name="psW", bufs=3, space="PSUM") as psW,
        ):
            # ---- constants into SBUF ----
            w1_sb = cpool.tile([d_in, d_hid], F32, tag="w1")
            nc.sync.dma_start(out=w1_sb[:], in_=w1[:])
            w2_sb = cpool.tile([d_hid, n_cls], F32, tag="w2")
            nc.sync.dma_start(out=w2_sb[:], in_=w2[:])
            b1_sb = cpool.tile([WINDOW, d_hid], F32, tag="b1")
            nc.sync.dma_start(out=b1_sb[:], in_=b1bc[:])
            b2_sb = cpool.tile([WINDOW, n_cls], F32, tag="b2")
            nc.sync.dma_start(out=b2_sb[:], in_=b2bc[:])
            iota_sb = cpool.tile([CHUNK, SBATCH * WINDOW], F32, tag="iota")
            nc.sync.dma_start(out=iota_sb[:], in_=iota[:])
            id_sb = cpool.tile([WINDOW, WINDOW], F32, tag="ident")
            nc.sync.dma_start(out=id_sb[:], in_=ident[:])
            idx_sb = cpool.tile([CHUNK, T * 8], I16, tag="idx")
            nc.sync.dma_start(out=idx_sb[:], in_=idx_t[:])
            dstrel_sb = cpool.tile([CHUNK, T], F32, tag="dstrel")
            nc.sync.dma_start(out=dstrel_sb[:], in_=dstrel_t[:])
            dinvw_sb = cpool.tile([WINDOW, w_cnt], F32, tag="dinvw")
            nc.sync.dma_start(out=dinvw_sb[:], in_=dinvw_t[:])

            zrow = cpool.tile([1, d_rep], BF16, tag="zrow")
            nc.vector.memset(zrow[:], 0.0)
            nc.sync.dma_start(out=h2tab[0:1, :], in_=zrow[:1, :])
            nc.sync.dma_start(out=h2tab[N + 1 : N + 2, :], in_=zrow[:1, :])

            def build_s(pool, t0, n, nm):
                """one-hot S for chunks [t0, t0+n) in one DVE op."""
                s_tile = pool.tile([CHUNK, SBATCH * WINDOW], BF16, tag="s", name=nm)
                rel_b = (
                    dstrel_sb[:, t0 : t0 + n]
                    .rearrange("p (b one) -> p b one", one=1)
                    .to_broadcast([CHUNK, n, WINDOW])
                )
                io_v = iota_sb[:, : n * WINDOW].rearrange("p (b j) -> p b j", j=WINDOW)
                s_v = s_tile[:, : n * WINDOW].rearrange("p (b j) -> p b j", j=WINDOW)
                nc.vector.tensor_tensor(
                    out=s_v, in0=io_v, in1=rel_b, op=mybir.AluOpType.is_equal
                )
                return s_tile

            # per-window accumulators in SBUF (LOW evicts, HIGH adds on top)
            aggT_sb = apool.tile([d_in, w_cnt * WINDOW], F32, tag="aggT")
            out2_sb = apool.tile([WINDOW, w_cnt * n_cls], F32, tag="out2")

            # =========================== PHASE A ===========================
            psum_of_win = {}
            for sec, t0, n, qn in groups:
                gb = gpool.tile([CHUNK, GSZ, d_in], dt_gat, tag="g", name="gb")
                nc.gpsimd.dma_gather(
                    gb[:, :n, :],
                    tab_view(xtab)[sec],
                    idx_sb[:, t0 * 8 : (t0 + n) * 8],
                    n * CHUNK,
                    n * CHUNK,
                    d_in,
                    single_packet=True,
                    queue_num=qn,
                )
                for bt0 in range(t0, t0 + n, SBATCH):
                    bn = min(SBATCH, t0 + n - bt0)
                    s_tile = build_s(spool, bt0, bn, "sA")
                    for t in range(bt0, bt0 + bn):
                        j = t - bt0
                        w, first, last, _sec = chunk_win[t]
                        if first:
                            psum_of_win[w] = psA.tile(
                                [d_in, WINDOW], F32, tag="agg", name="aggps"
                            )
                        nc.tensor.matmul(
                            out=psum_of_win[w][:],
                            lhsT=gb[:, t - t0, :],
                            rhs=s_tile[:, j * WINDOW : (j + 1) * WINDOW],
                            start=first,
                            stop=last,
                        )
                        if not last:
                            continue
                        ps = psum_of_win.pop(w)
                        wsl = aggT_sb[:, w * WINDOW : (w + 1) * WINDOW]
                        if _sec == 0:
                            nc.scalar.activation(out=wsl, in_=ps[:], func=Copy)
                        else:
                            nc.vector.tensor_tensor(
                                out=wsl, in0=ps[:], in1=wsl, op=mybir.AluOpType.add
                            )
                            _window_epilogue_A(
                                nc, w, wsl, wpool, psW, w1_sb, w2_sb, b1_sb,
                                dinvw_sb, id_sb, h2loc, n_local, d_in, d_hid,
                                n_cls, d_rep,
                            )

            # ======================= h2 exchange ==========================
            if n_cores > 1:
                nc.gpsimd.collective_compute(
                    "AllGather",
                    mybir.AluOpType.bypass,
                    replica_groups=[list(range(n_cores))],
                    ins=[h2loc[:]],
                    outs=[h2tab[1 : 1 + n_cores * n_local, :]],
                )
            else:
                nc.sync.dma_start(out=h2tab[1 : 1 + n_local, :], in_=h2loc[:])

            # =========================== PHASE B ===========================
            psum_of_win = {}
            for sec, t0, n, qn in groups:
                g2 = g2pool.tile([CHUNK, GSZ, d_rep], BF16, tag="g2", name="g2b")
                nc.gpsimd.dma_gather(
                    g2[:, :n, :],
                    tab_view(h2tab)[sec],
                    idx_sb[:, t0 * 8 : (t0 + n) * 8],
                    n * CHUNK,
                    n * CHUNK,
                    d_rep,
                    single_packet=True,
                    queue_num=qn,
                )
                for bt0 in range(t0, t0 + n, SBATCH):
                    bn = min(SBATCH, t0 + n - bt0)
                    s_tile = build_s(s2pool, bt0, bn, "sB")
                    for t in range(bt0, bt0 + bn):
                        j = t - bt0
                        w, first, last, _sec = chunk_win[t]
                        if first:
                            psum_of_win[w] = psA.tile(
                                [WINDOW, n_cls], F32, tag="agg", name="agg2ps"
                            )
                        nc.tensor.matmul(
                            out=psum_of_win[w][:],
                            lhsT=s_tile[:, j * WINDOW : (j + 1) * WINDOW],
                            rhs=g2[:, t - t0, :n_cls],
                            start=first,
                            stop=last,
                        )
                        if not last:
                            continue
                        ps = psum_of_win.pop(w)
                        osl = out2_sb[:, w * n_cls : (w + 1) * n_cls]
                        if _sec == 0:
                            nc.scalar.activation(out=osl, in_=ps[:], func=Copy)
                        else:
                            ob = wpool.tile([WINDOW, n_cls], F32, tag="ob")
                            nc.vector.tensor_tensor(
                                out=ob[:], in0=ps[:], in1=osl, op=mybir.AluOpType.add
                            )
                            ob2 = wpool.tile([WINDOW, n_cls], F32, tag="ob2")
                            nc.vector.tensor_scalar(
                                out=ob2[:],
                                in0=ob[:],
                                scalar1=dinvw_sb[:, w : w + 1],
                                scalar2=None,
                                op0=mybir.AluOpType.mult,
                            )
                            ob3 = wpool.tile([WINDOW, n_cls], F32, tag="ob3")
                            nc.vector.tensor_tensor(
                                out=ob3[:], in0=ob2[:], in1=b2_sb[:],
                                op=mybir.AluOpType.add,
                            )
                            nrows = min(WINDOW, n_local - w * WINDOW)
                            nc.sync.dma_start(
                                out=out_t[w * WINDOW : w * WINDOW + nrows, :],
                                in_=ob3[:nrows, :],
                            )

    nc.compile()
    return nc


def _window_epilogue_A(
    nc, w, aggT, wpool, psW, w1_sb, w2_sb, b1_sb, dinvw_sb, id_sb,
    h2loc, n_local, d_in, d_hid, n_cls, d_rep,
):
    """aggT [d_in, WINDOW] in SBUF -> replicated h2 rows in DRAM."""
    Relu = mybir.ActivationFunctionType.Relu
    Copy = mybir.ActivationFunctionType.Copy

    # h1 [dst, hid] = aggT.T @ W1
    h1_ps = psW.tile([WINDOW, d_hid], F32, tag="wps", name="h1_ps")
    nc.tensor.matmul(out=h1_ps[:], lhsT=aggT, rhs=w1_sb[:], start=True, stop=True)
    # scale by dinv[dst] (per-partition), + b1, relu
    r_sb = wpool.tile([WINDOW, d_hid], F32, tag="r")
    nc.vector.tensor_scalar(
        out=r_sb[:],
        in0=h1_ps[:],
        scalar1=dinvw_sb[:, w : w + 1],
        scalar2=None,
        op0=mybir.AluOpType.mult,
    )
    r2_sb = wpool.tile([WINDOW, d_hid], F32, tag="r2")
    nc.vector.tensor_tensor(
        out=r2_sb[:], in0=r_sb[:], in1=b1_sb[:], op=mybir.AluOpType.add
    )
    r3_sb = wpool.tile([WINDOW, d_hid], F32, tag="r3")
    nc.scalar.activation(out=r3_sb[:], in_=r2_sb[:], func=Relu)
    # transpose -> [hid, dst]
    rT_ps = psW.tile([d_hid, WINDOW], F32, tag="wps", name="rT_ps")
    nc.tensor.transpose(out=rT_ps[:], in_=r3_sb[:], identity=id_sb[:])
    rT_sb = wpool.tile([d_hid, WINDOW], F32, tag="rTs")
    nc.scalar.activation(out=rT_sb[:], in_=rT_ps[:], func=Copy)
    # h2 [dst, n_cls] = rT.T @ W2; scale by dinv[dst]; replicate REP x
    h2_ps = psW.tile([WINDOW, n_cls], F32, tag="wps", name="h2_ps")
    nc.tensor.matmul(out=h2_ps[:], lhsT=rT_sb[:], rhs=w2_sb[:], start=True, stop=True)
    h2_sb = wpool.tile([WINDOW, d_rep], BF16, tag="h2s")
    nc.vector.tensor_scalar(
        out=h2_sb[:].rearrange("p (r c) -> p r c", c=n_cls),
        in0=h2_ps[:]
        .rearrange("p (one c) -> p one c", one=1)
        .to_broadcast([WINDOW, REP, n_cls]),
        scalar1=dinvw_sb[:, w : w + 1],
        scalar2=None,
        op0=mybir.AluOpType.mult,
    )
    nrows = min(WINDOW, n_local - w * WINDOW)
    nc.sync.dma_start(
        out=h2loc[w * WINDOW : w * WINDOW + nrows, :], in_=h2_sb[:nrows, :]
    )


# --------------------------------------------------------------------------
# Entry point
# --------------------------------------------------------------------------
def _make_inputs(x, W1, b1, W2, b2, pp, dt_np, n_cores):
    N, d_in = x.shape
    NP = n_cores * pp["n_local"]
    W1 = np.asarray(W1, np.float32)
    b1 = np.asarray(b1, np.float32)
    W2 = np.asarray(W2, np.float32)
    b2 = np.asarray(b2, np.float32)
    d_hid = W1.shape[1]
    n_cls = W2.shape[1]
    xtab = np.zeros((NP + 2, d_in), np.float32)
    xtab[1 + pp["permpos"]] = x * pp["dinv"][:, None]
    xtab = xtab.astype(dt_np)
    iota_arr = np.broadcast_to(
        np.tile(np.arange(WINDOW, dtype=np.float32), SBATCH),
        (CHUNK, SBATCH * WINDOW),
    ).copy()
    shared = {
        "xtab": xtab,
        "w1": W1,
        "w2": W2,
        "b1bc": np.broadcast_to(b1, (WINDOW, d_hid)).astype(np.float32).copy(),
        "b2bc": np.broadcast_to(b2, (WINDOW, n_cls)).astype(np.float32).copy(),
        "iota": iota_arr,
        "ident": np.eye(WINDOW, dtype=np.float32),
    }
    in_maps = []
    for pc in pp["per_core"]:
        m = dict(shared)
        m["idx16"] = pc["idx16"]
        m["dstrel"] = pc["dstrel"]
        m["dinvw"] = pc["dinvw"]
        in_maps.append(m)
    return in_maps


def _run(x, edge_index, W1, b1, W2, b2, n_cores, trace=False):
    x = np.asarray(x, dtype=np.float32)
    N, d_in = x.shape
    d_hid = np.asarray(W1).shape[1]
    n_cls = np.asarray(W2).shape[1]
    assert d_in == 128 and d_hid == 128

    pp = _preprocess(x, edge_index, n_cores)
    dt_gat = BF16 if GATHER_BF16 else F32
    np_gat = np.dtype("bfloat16") if GATHER_BF16 else np.dtype("float32")

    nc = bacc.Bacc("TRN2", target_bir_lowering=False, debug=False, num_swdge_queues=4)
    _build(
        nc,
        N=n_cores * pp["n_local"],
        n_local=pp["n_local"],
        d_in=d_in,
        d_hid=d_hid,
        n_cls=n_cls,
        pp=pp,
        n_cores=n_cores,
        dt_gat=dt_gat,
    )

    import ml_dtypes  # noqa

    in_maps = _make_inputs(x, W1, b1, W2, b2, pp, np_gat, n_cores)
    res = run_bass_kernel_spmd(nc, in_maps, list(range(n_cores)), trace=trace)
    outs = [res.results[c]["out"] for c in range(n_cores)]
    full = np.concatenate(outs, axis=0)[pp["permpos"]]
    return full.astype(np.float32), res


def kernel(x, edge_index, W1, b1, W2, b2):
    out, _ = _run(x, edge_index, W1, b1, W2, b2, N_CORES)
    return out

